# revision 5
# baseline (speedup 1.0000x reference)
"""Trainium2 Bass kernel for nn_ActorCritic (GIN message passing + actor/critic).

Self-contained: hardcodes all shapes. Distributes the n_nodes^2 actor axis
across 8 NeuronCores (75 rows of the 600x600 action grid per core),
replicates node-embedding compute + weights, and does one tiny AllGather of
(local_max, local_sumexp) for the global softmax.
"""

import sys

for _p in ("/opt/trn_rl_repo", "/root/.axon_site/_ro/trn_rl_repo"):
    if _p not in sys.path:
        sys.path.insert(0, _p)

import numpy as np

import concourse.bass as bass
import concourse.bacc as bacc
import concourse.mybir as mybir
import concourse.bass_isa as bass_isa
import concourse.tile as tile
from concourse.bass_utils import run_bass_kernel_spmd

F32 = mybir.dt.float32
BF16 = mybir.dt.bfloat16
AF = mybir.ActivationFunctionType
OP = mybir.AluOpType
AX = mybir.AxisListType

N = 600           # nodes
IN_DIM = 2
H = 128           # hidden
HA = 256          # actor hidden
E = 9600
BN_EPS = 1e-5
NCORES = 8
IPC = N // NCORES  # 75 actor rows per core

# node-axis chunking (600 = 4*128 + 88)
PCS = [128, 128, 128, 128, 88]
POFF = [0, 128, 256, 384, 512]
NJB = len(PCS)
JH = [(0, 300), (300, 600)]  # matmul N<=512 chunks

# reduce dtype for the actor relu tiles / reduce matmuls
RD = BF16
# relu engine split: ACT every k-th (i,cb) tile, rest on DVE
ACT_EVERY = 4 if RD == BF16 else 3

_CACHE: dict = {}


def _build_nc():
    nc = bacc.Bacc("TRN2", target_bir_lowering=False, debug=False,
                   num_devices=NCORES)

    # ---------------- I/O declarations ----------------
    def din(name, shape):
        return nc.dram_tensor(name, list(shape), F32, kind="ExternalInput").ap()

    at_d = din("at", (N, N))          # AT'[j, i] = (A + I)[i, j]
    feat_d = din("feat", (N, IN_DIM))
    sel_d = din("sel", (N, IPC))      # per-core one-hot row selector
    g0W0_d = din("g0W0", (IN_DIM, H))
    g0W1_d = din("g0W1", (H, H))
    g1W0_d = din("g1W0", (H, H))
    g1W1_d = din("g1W1", (H, H))
    g0gam_d = din("g0gam", (H, 1))
    g0bet_d = din("g0bet", (H, 1))
    g1gam_d = din("g1gam", (H, 1))
    g1bet_d = din("g1bet", (H, 1))
    g0b1_d = din("g0b1", (H, 1))
    g1b1_d = din("g1b1", (H, 1))
    aW0_d = din("aW0", (3 * H, HA))
    ab0_d = din("ab0", (HA, 1))
    aW1_d = din("aW1", (HA, 1))
    cW0_d = din("cW0", (H, 2 * H))
    cb0_d = din("cb0", (2 * H, 1))
    cW1_d = din("cW1", (2 * H, 1))
    cb1_d = din("cb1", (1, 1))
    ident_d = din("ident", (H, H))

    pi_d = nc.dram_tensor("pi_out", [IPC, N], F32, kind="ExternalOutput").ap()
    val_d = nc.dram_tensor("value_out", [1, 1], F32, kind="ExternalOutput").ap()

    with tile.TileContext(nc) as tc:
        with tc.tile_pool(name="persist", bufs=1) as SP:
            # ---------------- load persistent SBUF data ----------------
            at_t = []
            feat_t = []
            sel_t = []
            for k in range(NJB):
                pc, off = PCS[k], POFF[k]
                a = SP.tile([pc, N], F32, name=f"at{k}")
                nc.sync.dma_start(a[:], at_d[off:off + pc, :])
                at_t.append(a)
                f = SP.tile([pc, IN_DIM], F32, name=f"feat{k}")
                nc.sync.dma_start(f[:], feat_d[off:off + pc, :])
                feat_t.append(f)
                s = SP.tile([pc, IPC], F32, name=f"sel{k}")
                nc.sync.dma_start(s[:], sel_d[off:off + pc, :])
                sel_t.append(s)

            def load(name, src, shape):
                t = SP.tile(list(shape), F32, name=name)
                nc.sync.dma_start(t[:], src)
                return t

            g0W0_t = load("g0W0t", g0W0_d[:], (IN_DIM, H))
            g0W1_t = load("g0W1t", g0W1_d[:], (H, H))
            g1W0_t = load("g1W0t", g1W0_d[:], (H, H))
            g1W1_t = load("g1W1t", g1W1_d[:], (H, H))
            g0gam_t = load("g0gamt", g0gam_d[:], (H, 1))
            g0bet_t = load("g0bett", g0bet_d[:], (H, 1))
            g1gam_t = load("g1gamt", g1gam_d[:], (H, 1))
            g1bet_t = load("g1bett", g1bet_d[:], (H, 1))
            g0b1_t = load("g0b1t", g0b1_d[:], (H, 1))
            g1b1_t = load("g1b1t", g1b1_d[:], (H, 1))
            ident_t = load("identt", ident_d[:], (H, H))
            Ws_t = [load(f"Ws{c}", aW0_d[0:H, c * H:(c + 1) * H], (H, H))
                    for c in range(2)]
            Wn1_t = [load(f"Wn1{c}", aW0_d[H:2 * H, c * H:(c + 1) * H], (H, H))
                     for c in range(2)]
            Wn2_t = [load(f"Wn2{c}", aW0_d[2 * H:3 * H, c * H:(c + 1) * H], (H, H))
                     for c in range(2)]
            ab0_t = [load(f"ab0{c}", ab0_d[c * H:(c + 1) * H, :], (H, 1))
                     for c in range(2)]
            aW1_t = [load(f"aW1{c}", aW1_d[c * H:(c + 1) * H, :], (H, 1))
                     for c in range(2)]
            cW0_t = [load(f"cW0{c}", cW0_d[:, c * H:(c + 1) * H], (H, H))
                     for c in range(2)]
            cb0_t = [load(f"cb0{c}", cb0_d[c * H:(c + 1) * H, :], (H, 1))
                     for c in range(2)]
            cW1_t = [load(f"cW1{c}", cW1_d[c * H:(c + 1) * H, :], (H, 1))
                     for c in range(2)]
            cb1_t = load("cb1t", cb1_d[:], (1, 1))

            # ---------------- GIN layers (replicated) ----------------
            def gin_layer(x_stat, Fin, W0_t, gam_t, bet_t, W1_t, b1_t, pp, lname):
                """x_stat: node-major tiles [pc, Fin]; returns feature-major
                hT [H, N] SBUF tile (Linear->BN->ReLU->Linear applied to
                (x + sum_neighbors x) per GINConv eps=0)."""
                # s = (A+I) @ x, feature-major: out[f, i] over K=j chunks
                sT = SP.tile([Fin, N], F32, name=f"sT_{lname}")
                for h, (j0, j1) in enumerate(JH):
                    sps = pp.tile([Fin, 300], F32, name="sps", tag="sps", bufs=2)
                    for k in range(NJB):
                        nc.tensor.matmul(sps[:], x_stat[k][:], at_t[k][:, j0:j1],
                                         start=(k == 0), stop=(k == NJB - 1))
                    nc.scalar.copy(sT[:, j0:j1], sps[:])
                # h1 = W0^T @ s  (feature-major [H, N])
                h1T = SP.tile([H, N], F32, name=f"h1T_{lname}")
                sums = []
                for h, (j0, j1) in enumerate(JH):
                    hps = pp.tile([H, 300], F32, name="hps", tag="hps", bufs=2)
                    nc.tensor.matmul(hps[:], W0_t[:], sT[:, j0:j1],
                                     start=True, stop=True)
                    acc = SP.tile([H, 1], F32, name=f"acc{h}_{lname}")
                    nc.scalar.activation(h1T[:, j0:j1], hps[:], AF.Identity,
                                         accum_out=acc[:])
                    sums.append(acc)
                # batch stats over nodes (free axis)
                s1 = SP.tile([H, 1], F32, name=f"s1_{lname}")
                nc.vector.tensor_add(s1[:], sums[0][:], sums[1][:])
                sqs = []
                for h, (j0, j1) in enumerate(JH):
                    sq = SP.tile([H, 300], F32, name=f"sq_{lname}", tag="sqscr")
                    acc2 = SP.tile([H, 1], F32, name=f"acc2{h}_{lname}")
                    nc.scalar.activation(sq[:], h1T[:, j0:j1], AF.Square,
                                         accum_out=acc2[:])
                    sqs.append(acc2)
                s2 = SP.tile([H, 1], F32, name=f"s2_{lname}")
                nc.vector.tensor_add(s2[:], sqs[0][:], sqs[1][:])
                mu = SP.tile([H, 1], F32, name=f"mu_{lname}")
                nc.vector.tensor_scalar_mul(mu[:], s1[:], 1.0 / N)
                ex2 = SP.tile([H, 1], F32, name=f"ex2_{lname}")
                nc.vector.tensor_scalar_mul(ex2[:], s2[:], 1.0 / N)
                mu2 = SP.tile([H, 1], F32, name=f"mu2_{lname}")
                nc.scalar.activation(mu2[:], mu[:], AF.Square)
                var = SP.tile([H, 1], F32, name=f"var_{lname}")
                nc.vector.tensor_sub(var[:], ex2[:], mu2[:])
                # rstd = exp(-0.5 * ln(var + eps))  (Ln+Exp share one ACT table set)
                epsc = SP.tile([H, 1], F32, name=f"epsc_{lname}")
                nc.vector.memset(epsc[:], BN_EPS)
                lnv = SP.tile([H, 1], F32, name=f"lnv_{lname}")
                nc.scalar.activation(lnv[:], var[:], AF.Ln, bias=epsc[:, 0:1])
                rstd = SP.tile([H, 1], F32, name=f"rstd_{lname}")
                nc.scalar.activation(rstd[:], lnv[:], AF.Exp, scale=-0.5)
                scl = SP.tile([H, 1], F32, name=f"scl_{lname}")
                nc.vector.tensor_mul(scl[:], gam_t[:], rstd[:])
                tmp = SP.tile([H, 1], F32, name=f"tmp_{lname}")
                nc.vector.tensor_mul(tmp[:], mu[:], scl[:])
                shf = SP.tile([H, 1], F32, name=f"shf_{lname}")
                nc.vector.tensor_sub(shf[:], bet_t[:], tmp[:])
                # y = relu(h1 * scl + shf), fused
                yT = SP.tile([H, N], F32, name=f"yT_{lname}")
                nc.scalar.activation(yT[:], h1T[:], AF.Relu,
                                     bias=shf[:, 0:1], scale=scl[:, 0:1])
                # h = W1^T @ y + b1
                hT = SP.tile([H, N], F32, name=f"hT_{lname}")
                for h, (j0, j1) in enumerate(JH):
                    ops = pp.tile([H, 300], F32, name="ops", tag="hps", bufs=2)
                    nc.tensor.matmul(ops[:], W1_t[:], yT[:, j0:j1],
                                     start=True, stop=True)
                    nc.scalar.activation(hT[:, j0:j1], ops[:], AF.Identity,
                                         bias=b1_t[:, 0:1])
                return hT

            def transpose_to_node_major(hT, pp, lname):
                hN = []
                for k in range(NJB):
                    pc, off = PCS[k], POFF[k]
                    tps = pp.tile([pc, H], F32, name="tps", tag="tps", bufs=2)
                    nc.tensor.transpose(tps[:], hT[:, off:off + pc], ident_t[:])
                    t = SP.tile([pc, H], F32, name=f"hN{k}_{lname}")
                    nc.scalar.copy(t[:], tps[:])
                    hN.append(t)
                return hN

            with tc.tile_pool(name="ppsum", bufs=1, space="PSUM") as pp:
                h0T = gin_layer(feat_t, IN_DIM, g0W0_t, g0gam_t, g0bet_t,
                                g0W1_t, g0b1_t, pp, "l0")
                h0N = transpose_to_node_major(h0T, pp, "l0")
                hT = gin_layer(h0N, H, g1W0_t, g1gam_t, g1bet_t,
                               g1W1_t, g1b1_t, pp, "l1")
                hN = transpose_to_node_major(hT, pp, "l1")

            # ---------------- pooling + critic + actor-pre ----------------
            geT = SP.tile([H, 1], F32, name="geT")
            ges = SP.tile([H, 1], F32, name="ges")
            nc.vector.tensor_reduce(ges[:], hT[:], axis=AX.X, op=OP.add)
            nc.vector.tensor_scalar_mul(geT[:], ges[:], 1.0 / N)

            base_t = [SP.tile([H, N], RD, name=f"base{c}") for c in range(2)]
            rowb_t = [SP.tile([H, IPC], F32, name=f"rowb{c}") for c in range(2)]
            w_t = [SP.tile([H, 1], RD, name=f"w{c}") for c in range(2)]

            with tc.tile_pool(name="bpsum", bufs=1, space="PSUM") as bp:
                # critic: value = relu(ge @ cW0 + cb0) @ cW1 + cb1
                u_sb = []
                for c in range(2):
                    ups = bp.tile([H, 1], F32, name="ups", tag="ups", bufs=1)
                    nc.tensor.matmul(ups[:], cW0_t[c][:], geT[:],
                                     start=True, stop=True)
                    u = SP.tile([H, 1], F32, name=f"u{c}")
                    nc.scalar.activation(u[:], ups[:], AF.Relu,
                                         bias=cb0_t[c][:, 0:1])
                    u_sb.append(u)
                vps = bp.tile([1, 1], F32, name="vps")
                nc.tensor.matmul(vps[:], cW1_t[0][:], u_sb[0][:],
                                 start=True, stop=False)
                nc.tensor.matmul(vps[:], cW1_t[1][:], u_sb[1][:],
                                 start=False, stop=True)
                val_sb = SP.tile([1, 1], F32, name="val_sb")
                nc.scalar.activation(val_sb[:], vps[:], AF.Identity,
                                     bias=cb1_t[0:1, 0:1])
                nc.sync.dma_start(val_d[:], val_sb[:])

                # h_loc = columns of hT for this core's rows (via sel one-hot)
                hlps = bp.tile([H, IPC], F32, name="hlps")
                for k in range(NJB):
                    nc.tensor.matmul(hlps[:], hN[k][:], sel_t[k][:],
                                     start=(k == 0), stop=(k == NJB - 1))
                hloc = SP.tile([H, IPC], F32, name="hloc")
                nc.scalar.copy(hloc[:], hlps[:])

                for c in range(2):
                    # sb = Ws^T @ ge  (+ ab0 folded into rowb below)
                    sps2 = bp.tile([H, 1], F32, name="sps2", tag="sps2", bufs=1)
                    nc.tensor.matmul(sps2[:], Ws_t[c][:], geT[:],
                                     start=True, stop=True)
                    sb = SP.tile([H, 1], F32, name=f"sb{c}")
                    nc.scalar.copy(sb[:], sps2[:])
                    # rowb = Wn2^T @ h_loc + sb + ab0
                    rps = bp.tile([H, IPC], F32, name="rps", tag="rps", bufs=1)
                    nc.tensor.matmul(rps[:], Wn2_t[c][:], hloc[:],
                                     start=True, stop=True)
                    nc.vector.tensor_scalar(rowb_t[c][:], rps[:],
                                            sb[:, 0:1], ab0_t[c][:, 0:1],
                                            op0=OP.add, op1=OP.add)
                    # base = Wn1^T @ h  (full 600 cols)
                    for hh, (j0, j1) in enumerate(JH):
                        bps = bp.tile([H, 300], F32, name="bps", tag="bps",
                                      bufs=2)
                        nc.tensor.matmul(bps[:], Wn1_t[c][:], hT[:, j0:j1],
                                         start=True, stop=True)
                        nc.scalar.copy(base_t[c][:, j0:j1], bps[:])
                    nc.scalar.copy(w_t[c][:], aW1_t[c][:])

            # ---------------- actor main loop ----------------
            with tc.tile_pool(name="PTpool", bufs=1, space="PSUM") as ptp, \
                 tc.tile_pool(name="cpsum", bufs=1, space="PSUM") as cp, \
                 tc.tile_pool(name="rpool", bufs=6) as rp, \
                 tc.tile_pool(name="smax", bufs=1) as sm:
                PT = [ptp.tile([PCS[k], IPC], F32, name=f"PT{k}")
                      for k in range(NJB)]
                for il in range(IPC):
                    for c in range(2):
                        R = rp.tile([H, N], RD, name="R", tag="R")
                        if ((il * 2 + c) % ACT_EVERY) == 0:
                            nc.scalar.activation(R[:], base_t[c][:], AF.Relu,
                                                 bias=rowb_t[c][:, il:il + 1])
                        else:
                            nc.vector.tensor_scalar(R[:], base_t[c][:],
                                                    rowb_t[c][:, il:il + 1],
                                                    0.0, op0=OP.add, op1=OP.max)
                        for k in range(NJB):
                            pc, off = PCS[k], POFF[k]
                            nc.tensor.matmul(PT[k][:, il:il + 1],
                                             R[:, off:off + pc], w_t[c][:],
                                             start=(c == 0), stop=(c == 1))

                # ---------------- softmax over all 360000 logits ----------
                L, mx, se, Ex = [], [], [], []
                for k in range(NJB):
                    pc = PCS[k]
                    Lk = sm.tile([pc, IPC], F32, name=f"L{k}")
                    nc.scalar.copy(Lk[:], PT[k][:])
                    L.append(Lk)
                    m = sm.tile([pc, 1], F32, name=f"mx{k}")
                    nc.vector.tensor_reduce(m[:], Lk[:], axis=AX.X, op=OP.max)
                    mx.append(m)
                # merge per-j maxima into one [128,1]
                m01 = sm.tile([H, 1], F32, name="m01")
                nc.vector.tensor_max(m01[:], mx[0][:], mx[1][:])
                m23 = sm.tile([H, 1], F32, name="m23")
                nc.vector.tensor_max(m23[:], mx[2][:], mx[3][:])
                m03 = sm.tile([H, 1], F32, name="m03")
                nc.vector.tensor_max(m03[:], m01[:], m23[:])
                m4p = sm.tile([H, 1], F32, name="m4p")
                nc.vector.memset(m4p[:], -3.0e38)
                nc.scalar.copy(m4p[0:88, :], mx[4][:])
                gmax = sm.tile([H, 1], F32, name="gmax")
                nc.vector.tensor_max(gmax[:], m03[:], m4p[:])
                # local max across partitions, broadcast to all
                mloc = sm.tile([H, 1], F32, name="mloc")
                nc.gpsimd.partition_all_reduce(mloc[:], gmax[:], channels=H,
                                               reduce_op=bass_isa.ReduceOp.max)
                nmloc = sm.tile([H, 1], F32, name="nmloc")
                nc.vector.tensor_scalar_mul(nmloc[:], mloc[:], -1.0)
                # E = exp(L - mloc), with per-row sums
                for k in range(NJB):
                    pc = PCS[k]
                    Ek = sm.tile([pc, IPC], F32, name=f"E{k}")
                    sk = sm.tile([pc, 1], F32, name=f"se{k}")
                    nc.scalar.activation(Ek[:], L[k][:], AF.Exp,
                                         bias=nmloc[0:pc, 0:1],
                                         accum_out=sk[:])
                    Ex.append(Ek)
                    se.append(sk)
                s01 = sm.tile([H, 1], F32, name="s01")
                nc.vector.tensor_add(s01[:], se[0][:], se[1][:])
                s23 = sm.tile([H, 1], F32, name="s23")
                nc.vector.tensor_add(s23[:], se[2][:], se[3][:])
                s03 = sm.tile([H, 1], F32, name="s03")
                nc.vector.tensor_add(s03[:], s01[:], s23[:])
                s4p = sm.tile([H, 1], F32, name="s4p")
                nc.vector.memset(s4p[:], 0.0)
                nc.scalar.copy(s4p[0:88, :], se[4][:])
                gsum = sm.tile([H, 1], F32, name="gsum")
                nc.vector.tensor_add(gsum[:], s03[:], s4p[:])
                ssum = sm.tile([H, 1], F32, name="ssum")
                nc.gpsimd.partition_all_reduce(ssum[:], gsum[:], channels=H,
                                               reduce_op=bass_isa.ReduceOp.add)

                # AllGather (m_c, S_c) from all 8 cores
                ccin_sb = sm.tile([1, 2], F32, name="ccin_sb")
                nc.scalar.copy(ccin_sb[0:1, 0:1], mloc[0:1, 0:1])
                nc.scalar.copy(ccin_sb[0:1, 1:2], ssum[0:1, 0:1])
                with tc.tile_pool(name="dram", bufs=1, space="DRAM") as dp:
                    ccin_d = dp.tile([1, 2], F32, name="ccin_d")
                    ccout_d = dp.tile([1, 2 * NCORES], F32, name="ccout_d",
                                      addr_space="Shared")
                    nc.sync.dma_start(ccin_d[:], ccin_sb[:])
                    nc.gpsimd.collective_compute(
                        "AllGather", OP.bypass,
                        replica_groups=[list(range(NCORES))],
                        ins=[ccin_d[:].opt()], outs=[ccout_d[:].opt()])
                    cc16 = sm.tile([1, 2 * NCORES], F32, name="cc16")
                    nc.sync.dma_start(cc16[:], ccout_d[:])

                ccv = cc16.rearrange("p (a b) -> p a b", b=2)
                mg = sm.tile([1, 1], F32, name="mg")
                nc.vector.tensor_reduce(mg[:], ccv[:, :, 0], axis=AX.X,
                                        op=OP.max)
                nmg = sm.tile([1, 1], F32, name="nmg")
                nc.vector.tensor_scalar_mul(nmg[:], mg[:], -1.0)
                e16 = sm.tile([1, 2 * NCORES], F32, name="e16")
                nc.scalar.activation(e16[:], cc16[:], AF.Exp,
                                     bias=nmg[0:1, 0:1])
                e16v = e16.rearrange("p (a b) -> p a b", b=2)
                p8 = sm.tile([1, NCORES], F32, name="p8")
                nc.vector.tensor_mul(p8[:], e16v[:, :, 0], ccv[:, :, 1])
                Sg = sm.tile([1, 1], F32, name="Sg")
                nc.vector.tensor_reduce(Sg[:], p8[:], axis=AX.X, op=OP.add)
                rec = sm.tile([1, 1], F32, name="rec")
                nc.vector.reciprocal(rec[:], Sg[:])
                em = sm.tile([1, 1], F32, name="em")
                nc.scalar.activation(em[:], mloc[0:1, 0:1], AF.Exp,
                                     bias=nmg[0:1, 0:1])
                fsc = sm.tile([1, 1], F32, name="fsc")
                nc.vector.tensor_mul(fsc[:], em[:], rec[:])
                fb = sm.tile([H, 1], F32, name="fb")
                nc.gpsimd.partition_broadcast(fb[:], fsc[:])

                # pi = E * f ; transpose to row-major [75, 600]; DMA out
                OUTt = sm.tile([IPC, N], F32, name="OUTt")
                for k in range(NJB):
                    pc, off = PCS[k], POFF[k]
                    Pk = sm.tile([pc, IPC], F32, name=f"P{k}")
                    nc.vector.tensor_scalar_mul(Pk[:], Ex[k][:], fb[0:pc, 0:1])
                    tp = cp.tile([IPC, H], F32, name="tp", tag="tp", bufs=2)
                    nc.tensor.transpose(tp[0:IPC, 0:pc], Pk[:],
                                        ident_t[0:pc, 0:pc])
                    nc.scalar.copy(OUTt[:, off:off + pc], tp[0:IPC, 0:pc])
                nc.sync.dma_start(pi_d[:], OUTt[:])

    nc.compile()
    return nc


def _get_nc():
    if "nc" not in _CACHE:
        _CACHE["nc"] = _build_nc()
    return _CACHE["nc"]


def _prep_inputs(inputs):
    f32 = lambda x: np.ascontiguousarray(np.asarray(x), dtype=np.float32)
    feat = f32(inputs["features"])
    ei = np.asarray(inputs["edge_index"]).astype(np.int64)
    src, dst = ei[0], ei[1]
    # AT'[j, i] = (A + I)[i, j]; A[i, j] = #edges j->i
    at = np.zeros((N, N), dtype=np.float32)
    np.add.at(at, (src, dst), 1.0)
    at[np.arange(N), np.arange(N)] += 1.0

    col = lambda x: f32(x).reshape(-1, 1)
    base = {
        "at": at,
        "feat": feat,
        "g0W0": f32(inputs["g0_W0"]), "g0W1": f32(inputs["g0_W1"]),
        "g1W0": f32(inputs["g1_W0"]), "g1W1": f32(inputs["g1_W1"]),
        "g0gam": col(inputs["g0_gamma"]), "g0bet": col(inputs["g0_beta"]),
        "g1gam": col(inputs["g1_gamma"]), "g1bet": col(inputs["g1_beta"]),
        "g0b1": col(inputs["g0_b1"]), "g1b1": col(inputs["g1_b1"]),
        "aW0": f32(inputs["a_W0"]), "ab0": col(inputs["a_b0"]),
        "aW1": f32(inputs["a_W1"]),
        "cW0": f32(inputs["c_W0"]), "cb0": col(inputs["c_b0"]),
        "cW1": f32(inputs["c_W1"]), "cb1": col(inputs["c_b1"]),
        "ident": np.eye(H, dtype=np.float32),
    }
    in_maps = []
    for c in range(NCORES):
        sel = np.zeros((N, IPC), dtype=np.float32)
        i0 = c * IPC
        sel[i0 + np.arange(IPC), np.arange(IPC)] = 1.0
        m = dict(base)
        m["sel"] = sel
        in_maps.append(m)
    return in_maps


def run(inputs, trace=False):
    nc = _get_nc()
    in_maps = _prep_inputs(inputs)
    res = run_bass_kernel_spmd(nc, in_maps, core_ids=list(range(NCORES)),
                               trace=trace)
    pi = np.concatenate(
        [np.asarray(res.results[c]["pi_out"], dtype=np.float32).reshape(-1)
         for c in range(NCORES)]).reshape(N * N, 1)
    value = np.asarray(res.results[0]["value_out"],
                       dtype=np.float32).reshape(1, 1)
    return (pi, value), res


def kernel(**inputs):
    out, _ = run(inputs)
    return out
